# revision 33
# baseline (speedup 1.0000x reference)
"""ChannelAttention Trainium2 kernel (Bass/Tile), data-parallel over batch.

Problem shapes (hardcoded):
  x      [8, 4096, 768] fp32
  w_qkv  [2304, 768]    fp32
  w_proj [768, 768]     fp32
  b_proj [768]          fp32
  out    [8, 4096, 768] fp32

Reference (per batch b, 8 groups of 96 channels):
  qkv = x @ w_qkv.T ; q *= N**-0.5
  attn_g = softmax(q_g.T @ k_g, axis=-1)     # [96, 96], contracts over N
  out_g  = attn_g @ v_g.T                    # [96, N]
  y = out @ w_proj.T + b_proj

Sharding: batch b -> core b (8 cores SPMD, no collectives).

Algebraic restructure (v4): channel attention collapses around two small
matrices --
  G = X^T X                      [768, 768]   (Gram, symmetric)
  attn_g = softmax(Wq_s G Wk^T)  (per group, [96, 96])
  M = Wv^T BD(attn)^T WprojT     [768, 768]
  y = x @ M + b_proj
so the per-token work is ONE 768-contraction pass for G (using x in
natural layout) and ONE for y (using x^T), plus O(768^3)-ish small
matmuls once per core. All matmul operands fp16 (full PE rate), fp32
accumulation in PSUM; softmax in fp32.
  - G accumulated over token supertiles; only upper 128-chunk blocks
    computed, lower mirrored by PE transpose (G symmetric).
  - x^T tiles produced by PE transpose of the natural x tiles (kept
    resident for the final y = x @ M pass).
  - P = BD(attn)^T WprojT per group (lhsT = softmax block directly),
    rows assembled into 128-aligned d-chunks via partition-shifting
    SBUF->SBUF DMA copies; M = Wv^T P with Wv in natural [d, a] layout.
Host pre-work: fp16 casts, fold N**-0.5 into Wq, transpose of the q/k
weight halves and of w_proj (layout prep only).
"""

import numpy as np

B, N, C = 8, 4096, 768
G = 8
GC = C // G          # 96
NCORES = 8
ST = 1024            # tokens per supertile
NST = N // ST        # 8
NSUB = ST // 128     # 4
CC = C // 128        # 6 chunks of the channel dim
QSCALE = float(N) ** -0.5  # 1/64

_CACHE = {}


def _build_nc():
    import concourse.bass as bass
    import concourse.mybir as mybir
    import concourse.tile as tile
    from concourse import bacc
    from concourse.masks import make_identity

    fp16 = mybir.dt.float16
    fp32 = mybir.dt.float32

    nc = bacc.Bacc(
        "TRN2", target_bir_lowering=False, debug=False, num_devices=NCORES
    )

    xh = nc.dram_tensor("xh", [N, C], fp16, kind="ExternalInput").ap()
    # q/k halves of w_qkv, transposed to [c, 2*768], q pre-scaled
    wqkT = nc.dram_tensor("wqkT", [C, 2 * C], fp16, kind="ExternalInput").ap()
    # v rows of w_qkv in natural [d, a] layout
    wv = nc.dram_tensor("wv", [C, C], fp16, kind="ExternalInput").ap()
    wprojT = nc.dram_tensor("wprojT", [C, C], fp16, kind="ExternalInput").ap()
    bproj = nc.dram_tensor("bproj", [C], fp32, kind="ExternalInput").ap()
    id16d = nc.dram_tensor("id16", [128, 128], fp16, kind="ExternalInput").ap()
    id32d = nc.dram_tensor("id32", [128, 128], fp32, kind="ExternalInput").ap()
    y = nc.dram_tensor("y", [N, C], fp32, kind="ExternalOutput").ap()

    with tile.TileContext(nc) as tc:
        from contextlib import ExitStack

        with ExitStack() as ctx:
            weights = ctx.enter_context(tc.tile_pool(name="weights", bufs=1))
            persist = ctx.enter_context(tc.tile_pool(name="persist", bufs=1))
            xn_pool = ctx.enter_context(tc.tile_pool(name="xn", bufs=22))
            ysb_pool = ctx.enter_context(tc.tile_pool(name="ysb", bufs=6))
            sm_pool = ctx.enter_context(tc.tile_pool(name="sm", bufs=4))
            ps_gram = ctx.enter_context(
                tc.tile_pool(name="ps_gram", bufs=4, space="PSUM")
            )
            ps_big = ctx.enter_context(
                tc.tile_pool(name="ps_big", bufs=4, space="PSUM")
            )

            # ---- static weight tiles (DMAs issued after phase 1 so the
            # token stream owns the HBM early) ----
            wqk_sb = [
                weights.tile([128, 2 * C], fp16, name=f"wqk_{a}")
                for a in range(CC)
            ]
            wv_sb = [
                weights.tile([128, C], fp16, name=f"wv_{dd}") for dd in range(CC)
            ]
            wpg_sb = [
                weights.tile([GC, C], fp16, name=f"wpg_{g}") for g in range(G)
            ]
            bias_sb = weights.tile([128, C], fp32, name="bias_sb")
            ident16 = weights.tile([128, 128], fp16, name="ident16")
            nc.scalar.dma_start(out=ident16, in_=id16d)
            ident32 = weights.tile([128, 128], fp32, name="ident32")
            nc.sync.dma_start(out=ident32, in_=id32d)

            # ---- persistent intermediates ----
            G_sb = [
                persist.tile([128, C], fp32, name=f"G_{a}") for a in range(CC)
            ]
            G16 = [
                persist.tile([128, C], fp16, name=f"G16_{a}") for a in range(CC)
            ]
            xT6 = [
                persist.tile([128, N], fp16, name=f"xT_{a}") for a in range(CC)
            ]
            e16 = [
                persist.tile([GC, GC], fp16, name=f"e16_{g}") for g in range(G)
            ]
            P6 = [persist.tile([128, C], fp16, name=f"P_{dd}") for dd in range(CC)]
            M_sb = [
                persist.tile([128, C], fp16, name=f"M_{a}") for a in range(CC)
            ]

            def gram_slices(a):
                out = []
                off = a * 128
                while off < C:
                    w = min(384, C - off)
                    out.append((off, w))
                    off += w
                return out

            # ---- phase 1: Gram accumulation + x^T materialization ----
            for s in range(NST):
                xn = []
                for t in range(NSUB):
                    xtile = xn_pool.tile(
                        [128, C], fp16, tag="xn", name=f"xn_{s}_{t}"
                    )
                    r0 = s * ST + t * 128
                    dma_eng = nc.scalar if t % 2 == 0 else nc.sync
                    dma_eng.dma_start(out=xtile, in_=xh[r0 : r0 + 128, :])
                    xn.append(xtile)

                for t in range(NSUB):
                    for a in range(CC):
                        tp_ps = ps_big.tile(
                            [128, 128], fp16, tag="big", name=f"tp_{s}_{t}_{a}"
                        )
                        nc.tensor.transpose(
                            tp_ps, xn[t][:, a * 128 : (a + 1) * 128], ident16
                        )
                        r0 = s * ST + t * 128
                        if (t * CC + a) % 3 == 0:
                            nc.scalar.copy(
                                out=xT6[a][:, r0 : r0 + 128], in_=tp_ps
                            )
                        else:
                            nc.vector.tensor_copy(
                                xT6[a][:, r0 : r0 + 128], tp_ps
                            )

                for a in range(CC):
                    for (off, w) in gram_slices(a):
                        g_ps = ps_gram.tile(
                            [128, 384], fp32, tag="gram", name=f"gps_{s}_{a}_{off}"
                        )
                        for t in range(NSUB):
                            nc.tensor.matmul(
                                g_ps[:, :w],
                                xn[t][:, a * 128 : (a + 1) * 128],
                                xn[t][:, off : off + w],
                                start=(t == 0),
                                stop=(t == NSUB - 1),
                            )
                        if s == 0:
                            nc.vector.tensor_copy(
                                G_sb[a][:, off : off + w], g_ps[:, :w]
                            )
                        else:
                            nc.vector.tensor_add(
                                G_sb[a][:, off : off + w],
                                G_sb[a][:, off : off + w],
                                g_ps[:, :w],
                            )

            # weight loads (needed from phase 2 on; scalar queue so the
            # sync queue stays in transpose mode for the xT loads)
            for a in range(CC):
                nc.scalar.dma_start(
                    out=wqk_sb[a], in_=wqkT[a * 128 : (a + 1) * 128, :]
                )
            for dd in range(CC):
                nc.scalar.dma_start(
                    out=wv_sb[dd], in_=wv[dd * 128 : (dd + 1) * 128, :]
                )
            for g in range(G):
                nc.scalar.dma_start(
                    out=wpg_sb[g], in_=wprojT[g * GC : (g + 1) * GC, :]
                )
            bias_bcast = bass.AP(
                tensor=bproj.tensor,
                offset=bproj.offset,
                ap=[[0, 128]] + [list(p) for p in bproj.ap],
            )
            nc.gpsimd.dma_start(out=bias_sb, in_=bias_bcast)

            # ---- phase 2a: mirror lower G blocks, cast each chunk as
            # soon as its mirrors land (shortens critical path into M1) ----
            for b_ in range(CC):
                for a in range(b_):
                    m_ps = ps_gram.tile(
                        [128, 128], fp32, tag="gram", name=f"mir_{a}_{b_}"
                    )
                    nc.tensor.transpose(
                        m_ps, G_sb[a][:, b_ * 128 : (b_ + 1) * 128], ident32
                    )
                    nc.vector.tensor_copy(
                        G_sb[b_][:, a * 128 : (a + 1) * 128], m_ps
                    )
                if b_ % 2 == 0:
                    nc.scalar.copy(out=G16[b_], in_=G_sb[b_])
                else:
                    nc.vector.tensor_copy(G16[b_], G_sb[b_])

            # ---- phase 2b: M1 = G Wk^T (all groups batched), then per
            # group A_g = Wq_s_g^T M1_g, softmax ----
            M1_sb = [
                persist.tile([128, C], fp16, name=f"m1_{a}") for a in range(CC)
            ]
            for a in range(CC):
                for half in range(2):
                    hsl = slice(half * 384, (half + 1) * 384)
                    m1_ps = ps_gram.tile(
                        [128, 384], fp32, tag="gram", name=f"m1ps_{a}_{half}"
                    )
                    for b_ in range(CC):
                        nc.tensor.matmul(
                            m1_ps,
                            G16[b_][:, a * 128 : (a + 1) * 128],
                            wqk_sb[b_][:, 768 + half * 384 : 768 + (half + 1) * 384],
                            start=(b_ == 0),
                            stop=(b_ == CC - 1),
                        )
                    if (a + half) % 2 == 0:
                        nc.scalar.copy(out=M1_sb[a][:, hsl], in_=m1_ps)
                    else:
                        nc.vector.tensor_copy(M1_sb[a][:, hsl], m1_ps)

            for g in range(G):
                a_ps = ps_big.tile([GC, GC], fp32, tag="big", name=f"aps_{g}")
                for a in range(CC):
                    nc.tensor.matmul(
                        a_ps,
                        wqk_sb[a][:, g * GC : (g + 1) * GC],
                        M1_sb[a][:, g * GC : (g + 1) * GC],
                        start=(a == 0),
                        stop=(a == CC - 1),
                    )

                nm = sm_pool.tile([GC, 1], fp32, tag="nm", name=f"nm_{g}")
                nc.vector.tensor_reduce(
                    out=nm,
                    in_=a_ps,
                    axis=mybir.AxisListType.X,
                    op=mybir.AluOpType.max,
                    negate=True,
                )
                e_t = sm_pool.tile([GC, GC], fp32, tag="e", name=f"e_{g}")
                ssum = sm_pool.tile([GC, 1], fp32, tag="ssum", name=f"ssum_{g}")
                nc.scalar.activation(
                    e_t,
                    a_ps,
                    mybir.ActivationFunctionType.Exp,
                    bias=nm,
                    scale=1.0,
                    accum_out=ssum,
                )
                rs = sm_pool.tile([GC, 1], fp32, tag="rs", name=f"rs_{g}")
                nc.vector.reciprocal(rs, ssum)
                nc.vector.tensor_scalar_mul(e16[g], e_t, rs)

            # ---- phase 2c: P = BD(attn)^T WprojT in 128-aligned d-chunks
            # (piece matmuls land at their global-d psum partitions via
            # tile_position col offsets) ; M = Wv^T P with K=128 ----
            def d_pieces(dd):
                raw = []
                for g in range(G):
                    lo, hi = g * GC, (g + 1) * GC
                    r0 = max(0, 128 * dd - lo)
                    r1 = min(GC, 128 * (dd + 1) - lo)
                    if r0 < r1:
                        raw.append((g, r0, r1, lo + r0 - 128 * dd))
                # split pieces that violate PE col-group placement rules
                # (M<=32 at {0,32,64,96}; M<=64 at {0,64}; M>64 only at 0)
                out = []
                for (g, r0, r1, p0) in raw:
                    while r0 < r1:
                        m = r1 - r0
                        if p0 == 0 or (m <= 32) or (m <= 64 and p0 == 64):
                            out.append((g, r0, r1, p0))
                            break
                        step = 32 if p0 % 64 else 64
                        step = min(step, m)
                        out.append((g, r0, r0 + step, p0))
                        r0 += step
                        p0 += step
                return out

            for dd in range(CC):
                for half in range(2):
                    hsl = slice(half * 384, (half + 1) * 384)
                    p_ps = ps_big.tile(
                        [128, 384], fp32, tag="big", name=f"pps_{dd}_{half}"
                    )
                    for (g, r0, r1, p0) in d_pieces(dd):
                        nc.tensor.matmul(
                            p_ps[p0 : p0 + (r1 - r0), :],
                            e16[g][:, r0:r1],
                            wpg_sb[g][:, hsl],
                            start=True,
                            stop=True,
                            tile_position=(0, p0) if p0 else None,
                        )
                    if dd % 2 == 0:
                        nc.scalar.copy(out=P6[dd][:, hsl], in_=p_ps)
                    else:
                        nc.vector.tensor_copy(P6[dd][:, hsl], p_ps)

            for half in range(2):
                for ab in range(CC):
                    hsl = slice(half * 384, (half + 1) * 384)
                    m_ps = ps_big.tile(
                        [128, 384], fp32, tag="big", name=f"mps_{ab}_{half}"
                    )
                    for dd in range(CC):
                        nc.tensor.matmul(
                            m_ps,
                            wv_sb[dd][:, ab * 128 : (ab + 1) * 128],
                            P6[dd][:, hsl],
                            start=(dd == 0),
                            stop=(dd == CC - 1),
                        )
                    if ab % 2 == 0:
                        nc.scalar.copy(out=M_sb[ab][:, hsl], in_=m_ps)
                    else:
                        nc.vector.tensor_copy(M_sb[ab][:, hsl], m_ps)

            # ---- phase 3: y = x @ M + b ----
            for s in range(NST):
                for t in range(NSUB):
                    r0 = s * ST + t * 128
                    y_sb = ysb_pool.tile(
                        [128, C], fp32, tag="ysb", name=f"ysb_{s}_{t}"
                    )
                    for half in range(2):
                        hsl = slice(half * 384, (half + 1) * 384)
                        y_ps = ps_big.tile(
                            [128, 384], fp32, tag="big", name=f"yps_{s}_{t}_{half}"
                        )
                        for a in range(CC):
                            nc.tensor.matmul(
                                y_ps,
                                xT6[a][:, r0 : r0 + 128],
                                M_sb[a][:, hsl],
                                start=(a == 0),
                                stop=(a == CC - 1),
                            )
                        nc.vector.tensor_add(y_sb[:, hsl], y_ps, bias_sb[:, hsl])
                    nc.scalar.dma_start(out=y[r0 : r0 + 128, :], in_=y_sb)

    nc.compile()
    return nc


def _get_nc():
    if "nc" not in _CACHE:
        _CACHE["nc"] = _build_nc()
    return _CACHE["nc"]


def _host_prep(x, w_qkv, w_proj, b_proj):
    x = np.asarray(x, dtype=np.float32)
    w_qkv = np.asarray(w_qkv, dtype=np.float32)
    w_proj = np.asarray(w_proj, dtype=np.float32)
    b_proj = np.asarray(b_proj, dtype=np.float32)

    wqk = w_qkv[: 2 * C, :].copy()
    wqk[:C, :] *= np.float32(QSCALE)
    wqkT_h = np.ascontiguousarray(wqk.T).astype(np.float16)       # [768, 1536]
    wv_h = np.ascontiguousarray(w_qkv[2 * C :, :]).astype(np.float16)
    wprojT_h = np.ascontiguousarray(w_proj.T).astype(np.float16)  # [768, 768]

    id16 = np.eye(128, dtype=np.float16)
    id32 = np.eye(128, dtype=np.float32)
    in_maps = []
    for b_ in range(NCORES):
        in_maps.append(
            {
                "xh": np.ascontiguousarray(x[b_]).astype(np.float16),
                "wqkT": wqkT_h,
                "wv": wv_h,
                "wprojT": wprojT_h,
                "bproj": b_proj,
                "id16": id16,
                "id32": id32,
            }
        )
    return in_maps


def _run(in_maps, trace=False):
    from concourse.bass_utils import run_bass_kernel_spmd

    nc = _get_nc()
    res = run_bass_kernel_spmd(nc, in_maps, list(range(NCORES)), trace=trace)
    out = np.stack([res.results[i]["y"] for i in range(NCORES)], axis=0)
    return out.astype(np.float32, copy=False), res


def kernel(x, w_qkv, w_proj, b_proj):
    in_maps = _host_prep(x, w_qkv, w_proj, b_proj)
    out, _ = _run(in_maps, trace=False)
    return out


def run_profiled(x, w_qkv, w_proj, b_proj):
    """Returns (out, BassKernelResults) with NTFF profiling enabled."""
    in_maps = _host_prep(x, w_qkv, w_proj, b_proj)
    return _run(in_maps, trace=True)


# revision 35
# speedup vs baseline: 1.0041x; 1.0041x over previous
"""ChannelAttention Trainium2 kernel (Bass/Tile), data-parallel over batch.

Problem shapes (hardcoded):
  x      [8, 4096, 768] fp32
  w_qkv  [2304, 768]    fp32
  w_proj [768, 768]     fp32
  b_proj [768]          fp32
  out    [8, 4096, 768] fp32

Reference (per batch b, 8 groups of 96 channels):
  qkv = x @ w_qkv.T ; q *= N**-0.5
  attn_g = softmax(q_g.T @ k_g, axis=-1)     # [96, 96], contracts over N
  out_g  = attn_g @ v_g.T                    # [96, N]
  y = out @ w_proj.T + b_proj

Sharding: batch b -> core b (8 cores SPMD, no collectives).

Algebraic restructure (v4): channel attention collapses around two small
matrices --
  G = X^T X                      [768, 768]   (Gram, symmetric)
  attn_g = softmax(Wq_s G Wk^T)  (per group, [96, 96])
  M = Wv^T BD(attn)^T WprojT     [768, 768]
  y = x @ M + b_proj
so the per-token work is ONE 768-contraction pass for G (using x in
natural layout) and ONE for y (using x^T), plus O(768^3)-ish small
matmuls once per core. All matmul operands fp16 (full PE rate), fp32
accumulation in PSUM; softmax in fp32.
  - G accumulated over token supertiles; only upper 128-chunk blocks
    computed, lower mirrored by PE transpose (G symmetric).
  - x^T tiles produced by PE transpose of the natural x tiles (kept
    resident for the final y = x @ M pass).
  - P = BD(attn)^T WprojT per group (lhsT = softmax block directly),
    rows assembled into 128-aligned d-chunks via partition-shifting
    SBUF->SBUF DMA copies; M = Wv^T P with Wv in natural [d, a] layout.
Host pre-work: fp16 casts, fold N**-0.5 into Wq, transpose of the q/k
weight halves and of w_proj (layout prep only).
"""

import numpy as np

B, N, C = 8, 4096, 768
G = 8
GC = C // G          # 96
NCORES = 8
ST = 1024            # tokens per supertile
NST = N // ST        # 8
NSUB = ST // 128     # 4
CC = C // 128        # 6 chunks of the channel dim
QSCALE = float(N) ** -0.5  # 1/64

_CACHE = {}


def _build_nc():
    import concourse.bass as bass
    import concourse.mybir as mybir
    import concourse.tile as tile
    from concourse import bacc
    from concourse.masks import make_identity

    fp16 = mybir.dt.float16
    fp32 = mybir.dt.float32

    nc = bacc.Bacc(
        "TRN2", target_bir_lowering=False, debug=False, num_devices=NCORES
    )

    xh = nc.dram_tensor("xh", [N, C], fp16, kind="ExternalInput").ap()
    # q/k halves of w_qkv, transposed to [c, 2*768], q pre-scaled
    wqkT = nc.dram_tensor("wqkT", [C, 2 * C], fp16, kind="ExternalInput").ap()
    # v rows of w_qkv in natural [d, a] layout
    wv = nc.dram_tensor("wv", [C, C], fp16, kind="ExternalInput").ap()
    wprojT = nc.dram_tensor("wprojT", [C, C], fp16, kind="ExternalInput").ap()
    bproj = nc.dram_tensor("bproj", [C], fp32, kind="ExternalInput").ap()
    id16d = nc.dram_tensor("id16", [128, 128], fp16, kind="ExternalInput").ap()
    id32d = nc.dram_tensor("id32", [128, 128], fp32, kind="ExternalInput").ap()
    y = nc.dram_tensor("y", [N, C], fp32, kind="ExternalOutput").ap()

    with tile.TileContext(nc) as tc:
        from contextlib import ExitStack

        with ExitStack() as ctx:
            weights = ctx.enter_context(tc.tile_pool(name="weights", bufs=1))
            persist = ctx.enter_context(tc.tile_pool(name="persist", bufs=1))
            xn_pool = ctx.enter_context(tc.tile_pool(name="xn", bufs=18))
            ysb_pool = ctx.enter_context(tc.tile_pool(name="ysb", bufs=6))
            sm_pool = ctx.enter_context(tc.tile_pool(name="sm", bufs=4))
            ps_gram = ctx.enter_context(
                tc.tile_pool(name="ps_gram", bufs=4, space="PSUM")
            )
            ps_big = ctx.enter_context(
                tc.tile_pool(name="ps_big", bufs=4, space="PSUM")
            )

            # ---- static weight tiles (DMAs issued after phase 1 so the
            # token stream owns the HBM early) ----
            wqk_sb = [
                weights.tile([128, 2 * C], fp16, name=f"wqk_{a}")
                for a in range(CC)
            ]
            wv_sb = [
                weights.tile([128, C], fp16, name=f"wv_{dd}") for dd in range(CC)
            ]
            wpg_sb = [
                weights.tile([GC, C], fp16, name=f"wpg_{g}") for g in range(G)
            ]
            bias_sb = weights.tile([128, C], fp32, name="bias_sb")
            ident16 = weights.tile([128, 128], fp16, name="ident16")
            nc.scalar.dma_start(out=ident16, in_=id16d)
            ident32 = weights.tile([128, 128], fp32, name="ident32")
            nc.sync.dma_start(out=ident32, in_=id32d)

            # ---- persistent intermediates ----
            G_sb = [
                persist.tile([128, C], fp16, name=f"G_{a}") for a in range(CC)
            ]
            xT6 = [
                persist.tile([128, N], fp16, name=f"xT_{a}") for a in range(CC)
            ]
            e16 = [
                persist.tile([GC, GC], fp16, name=f"e16_{g}") for g in range(G)
            ]
            P6 = [persist.tile([128, C], fp16, name=f"P_{dd}") for dd in range(CC)]
            M_sb = [
                persist.tile([128, C], fp16, name=f"M_{a}") for a in range(CC)
            ]

            def gram_slices(a):
                out = []
                off = a * 128
                while off < C:
                    w = min(384, C - off)
                    out.append((off, w))
                    off += w
                return out

            # ---- phase 1: Gram accumulation + x^T materialization ----
            for s in range(NST):
                xn = []
                for t in range(NSUB):
                    xtile = xn_pool.tile(
                        [128, C], fp16, tag="xn", name=f"xn_{s}_{t}"
                    )
                    r0 = s * ST + t * 128
                    dma_eng = nc.scalar if t % 2 == 0 else nc.sync
                    dma_eng.dma_start(out=xtile, in_=xh[r0 : r0 + 128, :])
                    xn.append(xtile)

                for t in range(NSUB):
                    for a in range(CC):
                        tp_ps = ps_big.tile(
                            [128, 128], fp16, tag="big", name=f"tp_{s}_{t}_{a}"
                        )
                        nc.tensor.transpose(
                            tp_ps, xn[t][:, a * 128 : (a + 1) * 128], ident16
                        )
                        r0 = s * ST + t * 128
                        if (t * CC + a) % 3 == 0:
                            nc.scalar.copy(
                                out=xT6[a][:, r0 : r0 + 128], in_=tp_ps
                            )
                        else:
                            nc.vector.tensor_copy(
                                xT6[a][:, r0 : r0 + 128], tp_ps
                            )

                for a in range(CC):
                    for (off, w) in gram_slices(a):
                        g_ps = ps_gram.tile(
                            [128, 384], fp32, tag="gram", name=f"gps_{s}_{a}_{off}"
                        )
                        for t in range(NSUB):
                            nc.tensor.matmul(
                                g_ps[:, :w],
                                xn[t][:, a * 128 : (a + 1) * 128],
                                xn[t][:, off : off + w],
                                start=(t == 0),
                                stop=(t == NSUB - 1),
                            )
                        if s == 0:
                            nc.vector.tensor_copy(
                                G_sb[a][:, off : off + w], g_ps[:, :w]
                            )
                        else:
                            nc.vector.tensor_add(
                                G_sb[a][:, off : off + w],
                                G_sb[a][:, off : off + w],
                                g_ps[:, :w],
                            )

            # weight loads (needed from phase 2 on; scalar queue so the
            # sync queue stays in transpose mode for the xT loads)
            for a in range(CC):
                nc.scalar.dma_start(
                    out=wqk_sb[a], in_=wqkT[a * 128 : (a + 1) * 128, :]
                )
            for dd in range(CC):
                nc.scalar.dma_start(
                    out=wv_sb[dd], in_=wv[dd * 128 : (dd + 1) * 128, :]
                )
            for g in range(G):
                nc.scalar.dma_start(
                    out=wpg_sb[g], in_=wprojT[g * GC : (g + 1) * GC, :]
                )
            bias_bcast = bass.AP(
                tensor=bproj.tensor,
                offset=bproj.offset,
                ap=[[0, 128]] + [list(p) for p in bproj.ap],
            )
            nc.gpsimd.dma_start(out=bias_sb, in_=bias_bcast)

            # ---- phase 2a: mirror lower G blocks, cast each chunk as
            # soon as its mirrors land (shortens critical path into M1) ----
            for b_ in range(CC):
                for a in range(b_):
                    m_ps = ps_gram.tile(
                        [128, 128], fp16, tag="gram", name=f"mir_{a}_{b_}"
                    )
                    nc.tensor.transpose(
                        m_ps, G_sb[a][:, b_ * 128 : (b_ + 1) * 128], ident16
                    )
                    if (a + b_) % 2 == 0:
                        nc.scalar.copy(
                            out=G_sb[b_][:, a * 128 : (a + 1) * 128], in_=m_ps
                        )
                    else:
                        nc.vector.tensor_copy(
                            G_sb[b_][:, a * 128 : (a + 1) * 128], m_ps
                        )

            # ---- phase 2b: M1 = G Wk^T (all groups batched), then per
            # group A_g = Wq_s_g^T M1_g, softmax ----
            M1_sb = [
                persist.tile([128, C], fp16, name=f"m1_{a}") for a in range(CC)
            ]
            for a in range(CC):
                for half in range(2):
                    hsl = slice(half * 384, (half + 1) * 384)
                    m1_ps = ps_gram.tile(
                        [128, 384], fp32, tag="gram", name=f"m1ps_{a}_{half}"
                    )
                    for b_ in range(CC):
                        nc.tensor.matmul(
                            m1_ps,
                            G_sb[b_][:, a * 128 : (a + 1) * 128],
                            wqk_sb[b_][:, 768 + half * 384 : 768 + (half + 1) * 384],
                            start=(b_ == 0),
                            stop=(b_ == CC - 1),
                        )
                    if (a + half) % 2 == 0:
                        nc.scalar.copy(out=M1_sb[a][:, hsl], in_=m1_ps)
                    else:
                        nc.vector.tensor_copy(M1_sb[a][:, hsl], m1_ps)

            for g in range(G):
                a_ps = ps_big.tile([GC, GC], fp32, tag="big", name=f"aps_{g}")
                for a in range(CC):
                    nc.tensor.matmul(
                        a_ps,
                        wqk_sb[a][:, g * GC : (g + 1) * GC],
                        M1_sb[a][:, g * GC : (g + 1) * GC],
                        start=(a == 0),
                        stop=(a == CC - 1),
                    )

                nm = sm_pool.tile([GC, 1], fp32, tag="nm", name=f"nm_{g}")
                nc.vector.tensor_reduce(
                    out=nm,
                    in_=a_ps,
                    axis=mybir.AxisListType.X,
                    op=mybir.AluOpType.max,
                    negate=True,
                )
                e_t = sm_pool.tile([GC, GC], fp32, tag="e", name=f"e_{g}")
                ssum = sm_pool.tile([GC, 1], fp32, tag="ssum", name=f"ssum_{g}")
                nc.scalar.activation(
                    e_t,
                    a_ps,
                    mybir.ActivationFunctionType.Exp,
                    bias=nm,
                    scale=1.0,
                    accum_out=ssum,
                )
                rs = sm_pool.tile([GC, 1], fp32, tag="rs", name=f"rs_{g}")
                nc.vector.reciprocal(rs, ssum)
                nc.vector.tensor_scalar_mul(e16[g], e_t, rs)

            # ---- phase 2c: P = BD(attn)^T WprojT in 128-aligned d-chunks
            # (piece matmuls land at their global-d psum partitions via
            # tile_position col offsets) ; M = Wv^T P with K=128 ----
            def d_pieces(dd):
                raw = []
                for g in range(G):
                    lo, hi = g * GC, (g + 1) * GC
                    r0 = max(0, 128 * dd - lo)
                    r1 = min(GC, 128 * (dd + 1) - lo)
                    if r0 < r1:
                        raw.append((g, r0, r1, lo + r0 - 128 * dd))
                # split pieces that violate PE col-group placement rules
                # (M<=32 at {0,32,64,96}; M<=64 at {0,64}; M>64 only at 0)
                out = []
                for (g, r0, r1, p0) in raw:
                    while r0 < r1:
                        m = r1 - r0
                        if p0 == 0 or (m <= 32) or (m <= 64 and p0 == 64):
                            out.append((g, r0, r1, p0))
                            break
                        step = 32 if p0 % 64 else 64
                        step = min(step, m)
                        out.append((g, r0, r0 + step, p0))
                        r0 += step
                        p0 += step
                return out

            for dd in range(CC):
                for half in range(2):
                    hsl = slice(half * 384, (half + 1) * 384)
                    p_ps = ps_big.tile(
                        [128, 384], fp32, tag="big", name=f"pps_{dd}_{half}"
                    )
                    for (g, r0, r1, p0) in d_pieces(dd):
                        nc.tensor.matmul(
                            p_ps[p0 : p0 + (r1 - r0), :],
                            e16[g][:, r0:r1],
                            wpg_sb[g][:, hsl],
                            start=True,
                            stop=True,
                            tile_position=(0, p0) if p0 else None,
                        )
                    if dd % 2 == 0:
                        nc.scalar.copy(out=P6[dd][:, hsl], in_=p_ps)
                    else:
                        nc.vector.tensor_copy(P6[dd][:, hsl], p_ps)

            for half in range(2):
                for ab in range(CC):
                    hsl = slice(half * 384, (half + 1) * 384)
                    m_ps = ps_big.tile(
                        [128, 384], fp32, tag="big", name=f"mps_{ab}_{half}"
                    )
                    for dd in range(CC):
                        nc.tensor.matmul(
                            m_ps,
                            wv_sb[dd][:, ab * 128 : (ab + 1) * 128],
                            P6[dd][:, hsl],
                            start=(dd == 0),
                            stop=(dd == CC - 1),
                        )
                    if ab % 2 == 0:
                        nc.scalar.copy(out=M_sb[ab][:, hsl], in_=m_ps)
                    else:
                        nc.vector.tensor_copy(M_sb[ab][:, hsl], m_ps)

            # ---- phase 3: y = x @ M + b ----
            for s in range(NST):
                for t in range(NSUB):
                    r0 = s * ST + t * 128
                    y_sb = ysb_pool.tile(
                        [128, C], fp32, tag="ysb", name=f"ysb_{s}_{t}"
                    )
                    for half in range(2):
                        hsl = slice(half * 384, (half + 1) * 384)
                        y_ps = ps_big.tile(
                            [128, 384], fp32, tag="big", name=f"yps_{s}_{t}_{half}"
                        )
                        for a in range(CC):
                            nc.tensor.matmul(
                                y_ps,
                                xT6[a][:, r0 : r0 + 128],
                                M_sb[a][:, hsl],
                                start=(a == 0),
                                stop=(a == CC - 1),
                            )
                        nc.vector.tensor_add(y_sb[:, hsl], y_ps, bias_sb[:, hsl])
                    nc.scalar.dma_start(out=y[r0 : r0 + 128, :], in_=y_sb)

    nc.compile()
    return nc


def _get_nc():
    if "nc" not in _CACHE:
        _CACHE["nc"] = _build_nc()
    return _CACHE["nc"]


def _host_prep(x, w_qkv, w_proj, b_proj):
    x = np.asarray(x, dtype=np.float32)
    w_qkv = np.asarray(w_qkv, dtype=np.float32)
    w_proj = np.asarray(w_proj, dtype=np.float32)
    b_proj = np.asarray(b_proj, dtype=np.float32)

    wqk = w_qkv[: 2 * C, :].copy()
    wqk[:C, :] *= np.float32(QSCALE)
    wqkT_h = np.ascontiguousarray(wqk.T).astype(np.float16)       # [768, 1536]
    wv_h = np.ascontiguousarray(w_qkv[2 * C :, :]).astype(np.float16)
    wprojT_h = np.ascontiguousarray(w_proj.T).astype(np.float16)  # [768, 768]

    id16 = np.eye(128, dtype=np.float16)
    id32 = np.eye(128, dtype=np.float32)
    in_maps = []
    for b_ in range(NCORES):
        in_maps.append(
            {
                "xh": np.ascontiguousarray(x[b_]).astype(np.float16),
                "wqkT": wqkT_h,
                "wv": wv_h,
                "wprojT": wprojT_h,
                "bproj": b_proj,
                "id16": id16,
                "id32": id32,
            }
        )
    return in_maps


def _run(in_maps, trace=False):
    from concourse.bass_utils import run_bass_kernel_spmd

    nc = _get_nc()
    res = run_bass_kernel_spmd(nc, in_maps, list(range(NCORES)), trace=trace)
    out = np.stack([res.results[i]["y"] for i in range(NCORES)], axis=0)
    return out.astype(np.float32, copy=False), res


def kernel(x, w_qkv, w_proj, b_proj):
    in_maps = _host_prep(x, w_qkv, w_proj, b_proj)
    out, _ = _run(in_maps, trace=False)
    return out


def run_profiled(x, w_qkv, w_proj, b_proj):
    """Returns (out, BassKernelResults) with NTFF profiling enabled."""
    in_maps = _host_prep(x, w_qkv, w_proj, b_proj)
    return _run(in_maps, trace=True)


# revision 37
# speedup vs baseline: 1.0062x; 1.0021x over previous
"""ChannelAttention Trainium2 kernel (Bass/Tile), data-parallel over batch.

Problem shapes (hardcoded):
  x      [8, 4096, 768] fp32
  w_qkv  [2304, 768]    fp32
  w_proj [768, 768]     fp32
  b_proj [768]          fp32
  out    [8, 4096, 768] fp32

Reference (per batch b, 8 groups of 96 channels):
  qkv = x @ w_qkv.T ; q *= N**-0.5
  attn_g = softmax(q_g.T @ k_g, axis=-1)     # [96, 96], contracts over N
  out_g  = attn_g @ v_g.T                    # [96, N]
  y = out @ w_proj.T + b_proj

Sharding: batch b -> core b (8 cores SPMD, no collectives).

Algebraic restructure (v4): channel attention collapses around two small
matrices --
  G = X^T X                      [768, 768]   (Gram, symmetric)
  attn_g = softmax(Wq_s G Wk^T)  (per group, [96, 96])
  M = Wv^T BD(attn)^T WprojT     [768, 768]
  y = x @ M + b_proj
so the per-token work is ONE 768-contraction pass for G (using x in
natural layout) and ONE for y (using x^T), plus O(768^3)-ish small
matmuls once per core. All matmul operands fp16 (full PE rate), fp32
accumulation in PSUM; softmax in fp32.
  - G accumulated over token supertiles; only upper 128-chunk blocks
    computed, lower mirrored by PE transpose (G symmetric).
  - x^T tiles produced by PE transpose of the natural x tiles (kept
    resident for the final y = x @ M pass).
  - P = BD(attn)^T WprojT per group (lhsT = softmax block directly),
    rows assembled into 128-aligned d-chunks via partition-shifting
    SBUF->SBUF DMA copies; M = Wv^T P with Wv in natural [d, a] layout.
Host pre-work: fp16 casts, fold N**-0.5 into Wq, transpose of the q/k
weight halves and of w_proj (layout prep only).
"""

import numpy as np

B, N, C = 8, 4096, 768
G = 8
GC = C // G          # 96
NCORES = 8
ST = 1024            # tokens per supertile
NST = N // ST        # 8
NSUB = ST // 128     # 4
CC = C // 128        # 6 chunks of the channel dim
QSCALE = float(N) ** -0.5  # 1/64

_CACHE = {}


def _build_nc():
    import concourse.bass as bass
    import concourse.mybir as mybir
    import concourse.tile as tile
    from concourse import bacc
    from concourse.masks import make_identity

    fp16 = mybir.dt.float16
    fp32 = mybir.dt.float32

    nc = bacc.Bacc(
        "TRN2", target_bir_lowering=False, debug=False, num_devices=NCORES
    )

    xh = nc.dram_tensor("xh", [N, C], fp16, kind="ExternalInput").ap()
    # q/k halves of w_qkv, transposed to [c, 2*768], q pre-scaled
    wqkT = nc.dram_tensor("wqkT", [C, 2 * C], fp16, kind="ExternalInput").ap()
    # v rows of w_qkv in natural [d, a] layout
    wv = nc.dram_tensor("wv", [C, C], fp16, kind="ExternalInput").ap()
    wprojT = nc.dram_tensor("wprojT", [C, C], fp16, kind="ExternalInput").ap()
    bproj = nc.dram_tensor("bproj", [C], fp32, kind="ExternalInput").ap()
    id16d = nc.dram_tensor("id16", [128, 128], fp16, kind="ExternalInput").ap()
    id32d = nc.dram_tensor("id32", [128, 128], fp32, kind="ExternalInput").ap()
    y = nc.dram_tensor("y", [N, C], fp32, kind="ExternalOutput").ap()

    with tile.TileContext(nc) as tc:
        from contextlib import ExitStack

        with ExitStack() as ctx:
            weights = ctx.enter_context(tc.tile_pool(name="weights", bufs=1))
            persist = ctx.enter_context(tc.tile_pool(name="persist", bufs=1))
            xn_pool = ctx.enter_context(tc.tile_pool(name="xn", bufs=18))
            xn0_pool = ctx.enter_context(tc.tile_pool(name="xn0", bufs=16))
            ysb_pool = ctx.enter_context(tc.tile_pool(name="ysb", bufs=6))
            sm_pool = ctx.enter_context(tc.tile_pool(name="sm", bufs=4))
            ps_gram = ctx.enter_context(
                tc.tile_pool(name="ps_gram", bufs=4, space="PSUM")
            )
            ps_big = ctx.enter_context(
                tc.tile_pool(name="ps_big", bufs=4, space="PSUM")
            )

            # ---- static weight tiles (DMAs issued after phase 1 so the
            # token stream owns the HBM early) ----
            wqk_sb = [
                weights.tile([128, 2 * C], fp16, name=f"wqk_{a}")
                for a in range(CC)
            ]
            wv_sb = [
                weights.tile([128, C], fp16, name=f"wv_{dd}") for dd in range(CC)
            ]
            wpg_sb = [
                weights.tile([GC, C], fp16, name=f"wpg_{g}") for g in range(G)
            ]
            bias_sb = weights.tile([128, C], fp32, name="bias_sb")
            ident16 = weights.tile([128, 128], fp16, name="ident16")
            nc.scalar.dma_start(out=ident16, in_=id16d)
            ident32 = weights.tile([128, 128], fp32, name="ident32")
            nc.sync.dma_start(out=ident32, in_=id32d)

            # ---- persistent intermediates ----
            G_sb = [
                persist.tile([128, C], fp32, name=f"G_{a}") for a in range(CC)
            ]
            G16 = [
                persist.tile([128, C], fp16, name=f"G16_{a}") for a in range(CC)
            ]
            xT6 = [
                persist.tile([128, N], fp16, name=f"xT_{a}") for a in range(CC)
            ]
            e16 = [
                persist.tile([GC, GC], fp16, name=f"e16_{g}") for g in range(G)
            ]
            P6 = [persist.tile([128, C], fp16, name=f"P_{dd}") for dd in range(CC)]
            M_sb = [
                persist.tile([128, C], fp16, name=f"M_{a}") for a in range(CC)
            ]

            def gram_slices(a, halved=False):
                out = []
                off = a * 128
                while off < C:
                    w = min(384, C - off)
                    if halved and off < 384 < off + w:
                        w = 384 - off
                    out.append((off, w))
                    off += w
                return out

            # ---- phase 1: Gram accumulation + x^T materialization ----
            for s in range(NST):
                xn = []
                if s == 0:
                    # supertile 0: column-halved tiles so the first
                    # transposes/gram start after half a tile lands
                    for t in range(NSUB):
                        xlo = xn0_pool.tile(
                            [128, 384], fp16, tag="xn0", name=f"xnl_{t}"
                        )
                        xhi = xn0_pool.tile(
                            [128, 384], fp16, tag="xn0", name=f"xnh_{t}"
                        )
                        r0 = t * 128
                        nc.scalar.dma_start(out=xlo, in_=xh[r0 : r0 + 128, 0:384])
                        nc.sync.dma_start(out=xhi, in_=xh[r0 : r0 + 128, 384:C])
                        xn.append((xlo, xhi))
                else:
                    for t in range(NSUB):
                        xtile = xn_pool.tile(
                            [128, C], fp16, tag="xn", name=f"xn_{s}_{t}"
                        )
                        r0 = s * ST + t * 128
                        dma_eng = nc.scalar if t % 2 == 0 else nc.sync
                        dma_eng.dma_start(out=xtile, in_=xh[r0 : r0 + 128, :])
                        xn.append(xtile)

                def xsl(t, off, w):
                    if s == 0:
                        lo, hi = xn[t]
                        if off < 384:
                            return lo[:, off : off + w]
                        return hi[:, off - 384 : off - 384 + w]
                    return xn[t][:, off : off + w]

                for t in range(NSUB):
                    for a in range(CC):
                        tp_ps = ps_big.tile(
                            [128, 128], fp16, tag="big", name=f"tp_{s}_{t}_{a}"
                        )
                        nc.tensor.transpose(
                            tp_ps, xsl(t, a * 128, 128), ident16
                        )
                        r0 = s * ST + t * 128
                        if (t * CC + a) % 3 == 0:
                            nc.scalar.copy(
                                out=xT6[a][:, r0 : r0 + 128], in_=tp_ps
                            )
                        else:
                            nc.vector.tensor_copy(
                                xT6[a][:, r0 : r0 + 128], tp_ps
                            )

                for a in range(CC):
                    for (off, w) in gram_slices(a, halved=(s == 0)):
                        g_ps = ps_gram.tile(
                            [128, 384], fp32, tag="gram", name=f"gps_{s}_{a}_{off}"
                        )
                        for t in range(NSUB):
                            nc.tensor.matmul(
                                g_ps[:, :w],
                                xsl(t, a * 128, 128),
                                xsl(t, off, w),
                                start=(t == 0),
                                stop=(t == NSUB - 1),
                            )
                        if s == 0:
                            nc.vector.tensor_copy(
                                G_sb[a][:, off : off + w], g_ps[:, :w]
                            )
                        else:
                            nc.vector.tensor_add(
                                G_sb[a][:, off : off + w],
                                G_sb[a][:, off : off + w],
                                g_ps[:, :w],
                            )

            # weight loads (needed from phase 2 on; scalar queue so the
            # sync queue stays in transpose mode for the xT loads)
            for a in range(CC):
                nc.scalar.dma_start(
                    out=wqk_sb[a], in_=wqkT[a * 128 : (a + 1) * 128, :]
                )
            for dd in range(CC):
                nc.scalar.dma_start(
                    out=wv_sb[dd], in_=wv[dd * 128 : (dd + 1) * 128, :]
                )
            for g in range(G):
                nc.scalar.dma_start(
                    out=wpg_sb[g], in_=wprojT[g * GC : (g + 1) * GC, :]
                )
            bias_bcast = bass.AP(
                tensor=bproj.tensor,
                offset=bproj.offset,
                ap=[[0, 128]] + [list(p) for p in bproj.ap],
            )
            nc.gpsimd.dma_start(out=bias_sb, in_=bias_bcast)

            # ---- phase 2a: mirror lower G blocks, cast each chunk as
            # soon as its mirrors land (shortens critical path into M1) ----
            for b_ in range(CC):
                for a in range(b_):
                    m_ps = ps_gram.tile(
                        [128, 128], fp32, tag="gram", name=f"mir_{a}_{b_}"
                    )
                    nc.tensor.transpose(
                        m_ps, G_sb[a][:, b_ * 128 : (b_ + 1) * 128], ident32
                    )
                    nc.vector.tensor_copy(
                        G_sb[b_][:, a * 128 : (a + 1) * 128], m_ps
                    )
                if b_ % 2 == 0:
                    nc.scalar.copy(out=G16[b_], in_=G_sb[b_])
                else:
                    nc.vector.tensor_copy(G16[b_], G_sb[b_])

            # ---- phase 2b: M1 = G Wk^T (all groups batched), then per
            # group A_g = Wq_s_g^T M1_g, softmax ----
            M1_sb = [
                persist.tile([128, C], fp16, name=f"m1_{a}") for a in range(CC)
            ]
            for a in range(CC):
                for half in range(2):
                    hsl = slice(half * 384, (half + 1) * 384)
                    m1_ps = ps_gram.tile(
                        [128, 384], fp32, tag="gram", name=f"m1ps_{a}_{half}"
                    )
                    for b_ in range(CC):
                        nc.tensor.matmul(
                            m1_ps,
                            G16[b_][:, a * 128 : (a + 1) * 128],
                            wqk_sb[b_][:, 768 + half * 384 : 768 + (half + 1) * 384],
                            start=(b_ == 0),
                            stop=(b_ == CC - 1),
                        )
                    if (a + half) % 2 == 0:
                        nc.scalar.copy(out=M1_sb[a][:, hsl], in_=m1_ps)
                    else:
                        nc.vector.tensor_copy(M1_sb[a][:, hsl], m1_ps)

            for g in range(G):
                a_ps = ps_big.tile([GC, GC], fp32, tag="big", name=f"aps_{g}")
                for a in range(CC):
                    nc.tensor.matmul(
                        a_ps,
                        wqk_sb[a][:, g * GC : (g + 1) * GC],
                        M1_sb[a][:, g * GC : (g + 1) * GC],
                        start=(a == 0),
                        stop=(a == CC - 1),
                    )

                nm = sm_pool.tile([GC, 1], fp32, tag="nm", name=f"nm_{g}")
                nc.vector.tensor_reduce(
                    out=nm,
                    in_=a_ps,
                    axis=mybir.AxisListType.X,
                    op=mybir.AluOpType.max,
                    negate=True,
                )
                e_t = sm_pool.tile([GC, GC], fp32, tag="e", name=f"e_{g}")
                ssum = sm_pool.tile([GC, 1], fp32, tag="ssum", name=f"ssum_{g}")
                nc.scalar.activation(
                    e_t,
                    a_ps,
                    mybir.ActivationFunctionType.Exp,
                    bias=nm,
                    scale=1.0,
                    accum_out=ssum,
                )
                rs = sm_pool.tile([GC, 1], fp32, tag="rs", name=f"rs_{g}")
                nc.vector.reciprocal(rs, ssum)
                nc.vector.tensor_scalar_mul(e16[g], e_t, rs)

            # ---- phase 2c: P = BD(attn)^T WprojT in 128-aligned d-chunks
            # (piece matmuls land at their global-d psum partitions via
            # tile_position col offsets) ; M = Wv^T P with K=128 ----
            def d_pieces(dd):
                raw = []
                for g in range(G):
                    lo, hi = g * GC, (g + 1) * GC
                    r0 = max(0, 128 * dd - lo)
                    r1 = min(GC, 128 * (dd + 1) - lo)
                    if r0 < r1:
                        raw.append((g, r0, r1, lo + r0 - 128 * dd))
                # split pieces that violate PE col-group placement rules
                # (M<=32 at {0,32,64,96}; M<=64 at {0,64}; M>64 only at 0)
                out = []
                for (g, r0, r1, p0) in raw:
                    while r0 < r1:
                        m = r1 - r0
                        if p0 == 0 or (m <= 32) or (m <= 64 and p0 == 64):
                            out.append((g, r0, r1, p0))
                            break
                        step = 32 if p0 % 64 else 64
                        step = min(step, m)
                        out.append((g, r0, r0 + step, p0))
                        r0 += step
                        p0 += step
                return out

            for dd in range(CC):
                for half in range(2):
                    hsl = slice(half * 384, (half + 1) * 384)
                    p_ps = ps_big.tile(
                        [128, 384], fp32, tag="big", name=f"pps_{dd}_{half}"
                    )
                    for (g, r0, r1, p0) in d_pieces(dd):
                        nc.tensor.matmul(
                            p_ps[p0 : p0 + (r1 - r0), :],
                            e16[g][:, r0:r1],
                            wpg_sb[g][:, hsl],
                            start=True,
                            stop=True,
                            tile_position=(0, p0) if p0 else None,
                        )
                    if dd % 2 == 0:
                        nc.scalar.copy(out=P6[dd][:, hsl], in_=p_ps)
                    else:
                        nc.vector.tensor_copy(P6[dd][:, hsl], p_ps)

            for half in range(2):
                for ab in range(CC):
                    hsl = slice(half * 384, (half + 1) * 384)
                    m_ps = ps_big.tile(
                        [128, 384], fp32, tag="big", name=f"mps_{ab}_{half}"
                    )
                    for dd in range(CC):
                        nc.tensor.matmul(
                            m_ps,
                            wv_sb[dd][:, ab * 128 : (ab + 1) * 128],
                            P6[dd][:, hsl],
                            start=(dd == 0),
                            stop=(dd == CC - 1),
                        )
                    if ab % 2 == 0:
                        nc.scalar.copy(out=M_sb[ab][:, hsl], in_=m_ps)
                    else:
                        nc.vector.tensor_copy(M_sb[ab][:, hsl], m_ps)

            # ---- phase 3: y = x @ M + b ----
            for s in range(NST):
                for t in range(NSUB):
                    r0 = s * ST + t * 128
                    y_sb = ysb_pool.tile(
                        [128, C], fp32, tag="ysb", name=f"ysb_{s}_{t}"
                    )
                    for half in range(2):
                        hsl = slice(half * 384, (half + 1) * 384)
                        y_ps = ps_big.tile(
                            [128, 384], fp32, tag="big", name=f"yps_{s}_{t}_{half}"
                        )
                        for a in range(CC):
                            nc.tensor.matmul(
                                y_ps,
                                xT6[a][:, r0 : r0 + 128],
                                M_sb[a][:, hsl],
                                start=(a == 0),
                                stop=(a == CC - 1),
                            )
                        nc.vector.tensor_add(y_sb[:, hsl], y_ps, bias_sb[:, hsl])
                    nc.scalar.dma_start(out=y[r0 : r0 + 128, :], in_=y_sb)

    nc.compile()
    return nc


def _get_nc():
    if "nc" not in _CACHE:
        _CACHE["nc"] = _build_nc()
    return _CACHE["nc"]


def _host_prep(x, w_qkv, w_proj, b_proj):
    x = np.asarray(x, dtype=np.float32)
    w_qkv = np.asarray(w_qkv, dtype=np.float32)
    w_proj = np.asarray(w_proj, dtype=np.float32)
    b_proj = np.asarray(b_proj, dtype=np.float32)

    wqk = w_qkv[: 2 * C, :].copy()
    wqk[:C, :] *= np.float32(QSCALE)
    wqkT_h = np.ascontiguousarray(wqk.T).astype(np.float16)       # [768, 1536]
    wv_h = np.ascontiguousarray(w_qkv[2 * C :, :]).astype(np.float16)
    wprojT_h = np.ascontiguousarray(w_proj.T).astype(np.float16)  # [768, 768]

    id16 = np.eye(128, dtype=np.float16)
    id32 = np.eye(128, dtype=np.float32)
    in_maps = []
    for b_ in range(NCORES):
        in_maps.append(
            {
                "xh": np.ascontiguousarray(x[b_]).astype(np.float16),
                "wqkT": wqkT_h,
                "wv": wv_h,
                "wprojT": wprojT_h,
                "bproj": b_proj,
                "id16": id16,
                "id32": id32,
            }
        )
    return in_maps


def _run(in_maps, trace=False):
    from concourse.bass_utils import run_bass_kernel_spmd

    nc = _get_nc()
    res = run_bass_kernel_spmd(nc, in_maps, list(range(NCORES)), trace=trace)
    out = np.stack([res.results[i]["y"] for i in range(NCORES)], axis=0)
    return out.astype(np.float32, copy=False), res


def kernel(x, w_qkv, w_proj, b_proj):
    in_maps = _host_prep(x, w_qkv, w_proj, b_proj)
    out, _ = _run(in_maps, trace=False)
    return out


def run_profiled(x, w_qkv, w_proj, b_proj):
    """Returns (out, BassKernelResults) with NTFF profiling enabled."""
    in_maps = _host_prep(x, w_qkv, w_proj, b_proj)
    return _run(in_maps, trace=True)
